# revision 50
# baseline (speedup 1.0000x reference)
"""Trainium2 Bass kernel for nn_MultiHeadAttention_39324720562623.

Reference computation (N=4, T=2048, D=512, H=8, HD=64), fp32:
    keys   = query @ Wk.T + query
    values = query @ Wv.T
    per head h: scores = softmax((Q_h @ K_h.T) / sqrt(HD))
                out_h  = scores @ V_h
    out = concat_heads(out_h) @ Wo.T

Sharding: 8 cores = 4 batches x 2 head-groups (4 heads each), pure SPMD.
Each core computes K/V projections for its head-group only, attention for
its 4 heads, and a partial O-projection; the host sums the two partials
per batch.  All operands are pre-transposed on the host so the kernel
works in "feature-major" (transposed) layouts throughout:

  qT   [512,2048]  query[n].T with feature dims permuted so this core's
                   head-group dims come first (bf16)
  wkiT [512, 256]  ((Wk + I)[hg,:][:,perm]).T  -> K.T = wkiT.T @ qT
                   (residual folded into the weight, bf16)
  wvT  [512, 256]  (Wv[hg,:][:,perm]).T        -> V = qT_chunk.T @ wvT
  woT  [256, 512]  as [64,4,512]-striped: Wo.T rows for this head-group
  outT [512,2048]  partial out.T (host sums pairs, then transposes)

All matmul operands are bf16 (1 cycle/row at any free width); PSUM
accumulation is fp32.  Scores PSUM is two rotating [128,1536] tiles (3
banks each); exp runs as width-1536 activation instructions to amortize
the per-instruction Act-engine overhead.  Softmax needs no
max-subtraction: scores/8 are bounded (|s|<~16) so exp is safe.  The
denominator comes for free from a ones-column appended to V (row 64 of
the att accumulation); normalization broadcasts the reciprocal
(reciprocal_approx_fast on DVE) across partitions with a sel-matrix
matmul on the PE.
"""

import os
import sys

for _p in ("/opt/trn_rl_repo", os.path.join(os.path.expanduser("~"), ".axon_site", "_ro", "trn_rl_repo")):
    if os.path.isdir(_p) and _p not in sys.path:
        sys.path.insert(0, _p)
        break

import numpy as np
import ml_dtypes

import concourse.bass as bass
import concourse.tile as tile
from concourse import mybir
from concourse.bass_utils import run_bass_kernel_spmd
from concourse.vector_clock import ScopedClock

N, T, D, H = 4, 2048, 512, 8
HD = D // H            # 64 head dim
HG = 2                 # head groups (cores per batch)
HPG = H // HG          # 4 heads per group
DG = D // HG           # 256 feature dims per group
F32 = mybir.dt.float32
BF16 = mybir.dt.bfloat16
EXP = mybir.ActivationFunctionType.Exp
MULT = mybir.AluOpType.mult

QT = 512               # q-tile width (matmul moving free dim)
NQT = T // QT          # 4
KC = 128               # k-chunk (partition dim of scores.T tiles)
NKC = T // KC          # 16
CH = 3                 # k-chunks per exp instruction / sc psum tile
SCW = CH * QT          # 1536 scores tile free width
NWARM = 12             # PE warmup matmuls (DVFS ramp during input DMA)


def _patch_drain():
    """walrus in this toolchain rejects >1 sync-wait on the kernel-tail
    Drain; split the waits across a chain of drains (1 wait each)."""
    if getattr(tile.TileContext, "_drain_split_patch", False):
        return

    def _drain_and_barrier(self, tick_clock, wait_clock):
        nc = self.nc
        d = nc.sync.drain()
        wait_clock.add_sem_waits(d.ins, ScopedClock({None: tick_clock.global_clock}))
        si = d.ins.sync_info
        waits = list(si.on_wait) if (si is not None and si.on_wait) else []
        if len(waits) > 1:
            si.on_wait = waits[:1]
            for w in waits[1:]:
                d2 = nc.sync.drain()
                if d2.ins.sync_info is None:
                    d2.ins.sync_info = mybir.SyncInfo(on_wait=[w], on_update=[])
                else:
                    d2.ins.sync_info.on_wait = [w]
        nc.all_engine_barrier()
        popped = nc._tile_sem_poison_stack.pop()
        assert popped is self._sem_poison
        nc.clear_and_free_semaphores(list(self.sems.allocated().values()))
        nc.all_engine_barrier()

    tile.TileContext._drain_and_barrier = _drain_and_barrier
    tile.TileContext._drain_split_patch = True


MAX_WAITS = 1


def _split_excess_waits(nc, maxw=MAX_WAITS):
    """walrus codegen rejects instructions with more than ~2 sync waits.
    Hoist excess waits onto same-engine nops inserted immediately before
    the offending instruction (same engine-stream position => identical
    semantics)."""
    nid = [0]

    def mk_nop(engine, waits):
        nid[0] += 1
        nop = mybir.InstNoOp(name=f"I-waitsplit-{nid[0]}")
        nop.engine = engine
        nop.sync_info = mybir.SyncInfo(on_wait=list(waits), on_update=[])
        try:
            nop.bass_nofuse = True
        except Exception:
            pass
        return nop

    for f in nc.m.functions:
        for bb in f.blocks:
            insts = bb.instructions
            i = 0
            while i < len(insts):
                ins = insts[i]
                si = ins.sync_info
                waits = list(si.on_wait) if (si is not None and si.on_wait) else []
                if len(waits) > maxw:
                    si.on_wait = waits[-maxw:]
                    excess = waits[:-maxw]
                    pos = i
                    for j in range(0, len(excess), maxw):
                        insts.insert(pos, mk_nop(ins.engine, excess[j : j + maxw]))
                        pos += 1
                        i += 1
                i += 1


def build_program(split_waits=True):
    _patch_drain()
    nc = bass.Bass()
    qT = nc.dram_tensor("qT", [D, T], BF16, kind="ExternalInput")
    wkiT = nc.dram_tensor("wkiT", [D, DG], BF16, kind="ExternalInput")
    wvT = nc.dram_tensor("wvT", [D, DG], BF16, kind="ExternalInput")
    woT = nc.dram_tensor("woT", [HPG * 128, D], BF16, kind="ExternalInput")
    zeros = nc.dram_tensor("zeros", [64, T], BF16, kind="ExternalInput")
    sel = nc.dram_tensor("sel", [4, HPG * 64], BF16, kind="ExternalInput")
    outT = nc.dram_tensor("outT", [D, T], F32, kind="ExternalOutput")

    with tile.TileContext(nc) as tc:
        with (
            tc.tile_pool(name="singles", bufs=1) as singles,
            tc.tile_pool(name="pt", bufs=3) as ptp,
            tc.tile_pool(name="attu", bufs=6) as attup,
            tc.tile_pool(name="denq", bufs=2) as denqp,
            tc.tile_pool(name="recq", bufs=2) as recqp,
            tc.tile_pool(name="denb", bufs=4) as denbp,
            tc.tile_pool(name="outp", bufs=4) as outp,
            tc.tile_pool(name="ps_sc", bufs=2, space="PSUM") as ps_sc,
            tc.tile_pool(name="ps_att", bufs=2, space="PSUM") as ps_att,
        ):
            qT_sb = singles.tile([128, 4, T], BF16)
            wkiT_sb = singles.tile([128, 4, DG], BF16)
            wvT_sb = singles.tile([128, 4, DG], BF16)
            woT_sb = singles.tile([128, HPG, D], BF16)
            sel_sb = singles.tile([4, HPG, 64], BF16)
            onescol = singles.tile([1, 64], BF16)
            warm_sb = singles.tile([128, QT], BF16)
            kT_pad = singles.tile([128, HPG, T], BF16)
            # V augmented with a ones column per head: [128, kchunk, head, 65]
            vaug_sb = singles.tile([128, NKC, HPG, HD + 1], BF16)
            attT_sb = singles.tile([128, HPG, T], BF16)

            # ---- PE warmup (no DMA dependency): ramp the DVFS clock while
            # inputs stream in.  memset runs on gpsimd at t=0.
            # warm_sb feeds PE-warming matmuls whose output is never read;
            # fill it via DMA (lands ~1.5us in) rather than an engine memset
            # that would queue behind the engine preamble.
            nc.sync.dma_start(warm_sb[0:64, :], zeros[:, 0:QT])
            nc.sync.dma_start(warm_sb[64:128, :], zeros[:, 0:QT])
            nc.gpsimd.memset(onescol[:], 1.0)

            def junk_mm(n, tag):
                for w in range(n):
                    wps = ps_sc.tile([128, SCW], F32, tag="sc", name=f"{tag}{w}")
                    nc.tensor.matmul(
                        wps[:, 0:QT],
                        warm_sb[:, 0:128],
                        warm_sb[:],
                        start=True,
                        stop=True,
                    )
            junk_mm(NWARM, "warm")

            # ---- input DMAs (fine-grained so compute can start early) ----
            nc.sync.dma_start(wkiT_sb[:], wkiT.rearrange("(c p) d -> p c d", p=128))
            # qT arrives time-slice-major: all 4 feature chunks of tt=0
            # first, so K-proj tile 0 can start after ~0.5MiB.  wvT lands
            # right after tt=0 so the V projection is never DMA-blocked.
            qT_r = qT.rearrange("(c p) (s t) -> c p s t", p=128, t=QT)
            for c in range(4):
                nc.sync.dma_start(qT_sb[:, c, 0:QT], qT_r[c, :, 0])
            nc.sync.dma_start(wvT_sb[:], wvT.rearrange("(c p) d -> p c d", p=128))
            for tt in range(1, NQT):
                for c in range(4):
                    nc.sync.dma_start(
                        qT_sb[:, c, QT * tt : QT * (tt + 1)], qT_r[c, :, tt]
                    )
            nc.sync.dma_start(woT_sb[:], woT.rearrange("(c p) d -> p c d", p=128))
            nc.sync.dma_start(sel_sb[:], sel.rearrange("p (h d) -> p h d", d=64))
            nc.gpsimd.memset(vaug_sb[:, :, :, HD], 1.0)
            for h in range(HPG):
                off = 64 - (h % 2) * 64  # complement of the head's parity slot
                nc.sync.dma_start(kT_pad[off : off + 64, h], zeros[:])
                nc.sync.dma_start(attT_sb[64:128, h], zeros[:])

            # ---- K.T = (Wk+I) @ q.T for this head-group: [256, 2048] ----
            # tt-outer so each tile only needs qT time-slice tt.
            for tt in range(NQT):
                for dc in range(2):
                    ps = ps_sc.tile([128, SCW], F32, tag="sc", name=f"ktps{dc}_{tt}")[
                        :, 0:QT
                    ]
                    for di in range(4):
                        nc.tensor.matmul(
                            ps[:],
                            wkiT_sb[:, di, 128 * dc : 128 * (dc + 1)],
                            qT_sb[:, di, QT * tt : QT * (tt + 1)],
                            start=(di == 0),
                            stop=(di == 3),
                        )
                    tsl = slice(QT * tt, QT * (tt + 1))
                    nc.vector.tensor_copy(kT_pad[0:64, 2 * dc, tsl], ps[0:64, :])
                    nc.vector.tensor_copy(
                        kT_pad[64:128, 2 * dc + 1, tsl], ps[64:128, :]
                    )

            # ---- V = q @ Wv.T for this head-group: [2048, 256] ----
            for tci in range(NKC):
                ps = ps_sc.tile([128, SCW], F32, tag="sc", name=f"vps{tci}")[:, 0:DG]
                for di in range(4):
                    nc.tensor.matmul(
                        ps[:],
                        qT_sb[:, di, 128 * tci : 128 * (tci + 1)],
                        wvT_sb[:, di],
                        start=(di == 0),
                        stop=(di == 3),
                    )
                nc.vector.tensor_copy(
                    vaug_sb[:, tci, :, 0:HD],
                    ps.rearrange("p (h d) -> p h d", d=HD),
                )

            # ---- attention: one global chunk stream over (jq, h, ik) ----
            def emit_oproj_dt(jqo, dt):
                osl = slice(QT * jqo, QT * (jqo + 1))
                ps = ps_sc.tile([128, SCW], F32, tag="sc", name=f"ops{jqo}_{dt}")[
                    :, 0:QT
                ]
                for hc in range(HPG):
                    nc.tensor.matmul(
                        ps[:],
                        woT_sb[:, hc, 128 * dt : 128 * (dt + 1)],
                        attT_sb[:, hc, osl],
                        start=(hc == 0),
                        stop=(hc == HPG - 1),
                    )
                ot = outp.tile([128, QT], F32, tag="ot", name=f"ot{jqo}_{dt}")
                nc.vector.tensor_copy(ot[:], ps[:])
                nc.sync.dma_start(outT[128 * dt : 128 * (dt + 1), osl], ot[:])

            # --- per-q-tile batched normalization state ---
            att_un = {}    # (jq, h) -> unnormalized head in SBUF f32
            denq_t = {}    # jq -> [4, QT] denominator collector (DMA gather)
            rec4_t = {}    # jq -> [4, QT] bf16 reciprocals
            rec1_t = {}    # (jq, h) -> [1, QT] bf16 reciprocal (last q-tile)

            def arm_norm(jq, h):
                # copy the unnormalized head (incl. denominator row 64) to
                # SBUF; DMA row 64 into the per-q-tile collector so one
                # reciprocal serves all 4 heads.  The last q-tile instead
                # runs a per-head chain immediately — its reciprocals overlap
                # the remaining stream instead of serializing into the tail.
                att_ps, _ = att_tiles[(jq, h)]
                au = attup.tile([HD + 1, QT], F32, tag="attu", name=f"au{jq}_{h}")
                nc.vector.tensor_copy(au[:], att_ps[:])
                att_un[(jq, h)] = au
                if jq == NQT - 1:
                    # issue the reciprocal now (DVE-only); the PE-side
                    # broadcast+multiply comes later via finish_direct.
                    rq = recqp.tile([1, QT], F32, tag="recd", name=f"rqd{jq}_{h}")
                    nc.vector.reciprocal(rq[:], au[HD : HD + 1, :])
                    r1 = recqp.tile([1, QT], BF16, tag="recd1", name=f"r1d{jq}_{h}")
                    with nc.allow_low_precision(
                        reason="bf16 reciprocal; softmax scale error ~0.4%"
                    ):
                        nc.vector.tensor_copy(r1[:], rq[:])
                    rec1_t[(jq, h)] = r1
                    return
                if jq not in denq_t:
                    denq_t[jq] = denqp.tile([4, QT], F32, tag="denq", name=f"dq{jq}")
                nc.sync.dma_start(denq_t[jq][h : h + 1, :], au[HD : HD + 1, :])

            def finish_direct(jq, h):
                # contraction-1 broadcast of the per-head reciprocal row
                qsl = slice(QT * jq, QT * (jq + 1))
                au = att_un[(jq, h)]
                rec_ps = ps_sc.tile([128, SCW], F32, tag="sc", name=f"rpd{jq}_{h}")[
                    0:64, 0:QT
                ]
                nc.tensor.matmul(
                    rec_ps[:], onescol[:], rec1_t[(jq, h)][:], start=True, stop=True
                )
                rec_bc = denbp.tile([64, QT], BF16, tag="denb", name=f"rbd{jq}_{h}")
                with nc.allow_low_precision(
                    reason="bf16 attT feeds the bf16 O-projection matmul"
                ):
                    nc.vector.tensor_copy(rec_bc[:], rec_ps[:])
                    nc.vector.tensor_tensor(
                        attT_sb[0:64, h, qsl], au[0:HD, :], rec_bc[:], MULT
                    )

            def emit_recip(jq):
                rq = recqp.tile([4, QT], F32, tag="recq", name=f"rq{jq}")
                nc.vector.reciprocal(rq[:], denq_t[jq][:])
                r4 = recqp.tile([4, QT], BF16, tag="rec4", name=f"r4{jq}")
                with nc.allow_low_precision(
                    reason="bf16 reciprocal; softmax scale error ~0.4%"
                ):
                    nc.vector.tensor_copy(r4[:], rq[:])
                rec4_t[jq] = r4

            def emit_norm_head(jq, h):
                qsl = slice(QT * jq, QT * (jq + 1))
                rec_ps = ps_sc.tile([128, SCW], F32, tag="sc", name=f"rp{jq}_{h}")[
                    0:64, 0:QT
                ]
                nc.tensor.matmul(
                    rec_ps[:], sel_sb[:, h], rec4_t[jq][:], start=True, stop=True
                )
                rec_bc = denbp.tile([64, QT], BF16, tag="denb", name=f"rb{jq}_{h}")
                with nc.allow_low_precision(
                    reason="bf16 attT feeds the bf16 O-projection matmul"
                ):
                    nc.vector.tensor_copy(rec_bc[:], rec_ps[:])
                    nc.vector.tensor_tensor(
                        attT_sb[0:64, h, qsl],
                        att_un[(jq, h)][0:HD, :],
                        rec_bc[:],
                        MULT,
                    )

            # software-pipelined chunk stream: per steady-state cycle the PE
            # does [3 score fills of tile k][3 AV matmuls of tile k-1] with
            # O-projection pieces and the deferred norm interleaved, while
            # the Act engine exps tile k.  Tiles flow across head
            # boundaries so every exp instruction is full width.
            stream = [
                (jq, h, ik)
                for jq in range(NQT)
                for h in range(HPG)
                for ik in range(NKC)
            ]
            cur = {"sc": None, "pt": None, "chunks": []}
            att_tiles = {}
            deferred = []  # (pt_tile, chunks) with exp issued, AV pending

            def emit_avs():
                pt_t, chunks = deferred.pop(0)
                for j, (jq, h, ik) in enumerate(chunks):
                    att_ps, nmm = att_tiles[(jq, h)]
                    nc.tensor.matmul(
                        att_ps[:],
                        vaug_sb[:, ik, h],
                        pt_t[:, QT * j : QT * (j + 1)],
                        start=(nmm == 0),
                        stop=(nmm == NKC - 1),
                    )
                    att_tiles[(jq, h)][1] = nmm + 1
                    if ik == NKC - 1:
                        arm_norm(jq, h)

            boundary_q = []  # norm/O-proj work queued to tile boundaries

            def close_tile():
                n = len(cur["chunks"])
                if n == 0:
                    return
                w = QT * n
                nc.scalar.activation(
                    cur["pt"][:, 0:w], cur["sc"][:, 0:w], EXP, scale=0.125
                )
                deferred.append((cur["pt"], cur["chunks"]))
                cur["sc"] = None
                cur["pt"] = None
                cur["chunks"] = []
                # AV runs two tiles behind its exp so the PE never races the
                # Act engine.
                if len(deferred) >= 3:
                    emit_avs()
                # boundary work (norm broadcasts, O-projection pieces) is
                # emitted only between score tiles so the three fills of a
                # tile are always consecutive on the PE — a split fill
                # delays the exp and hiccups the Act pipeline.
                while boundary_q:
                    boundary_q.pop(0)()

            # normalization/O-projection interleave points (all for q-tile
            # jq-1's norm, emitted during q-tile jq; O pieces for jq-1 are
            # emitted during jq at h2/h3, after its attT completes at h1):
            #   (h0, ik12): reciprocal (all 4 arms of jq-1 landed by ~h0,ik7)
            #   (h1, ik 0/4/8/12): per-head broadcast+multiply
            #   (h2, ik 0/8) and (h3, ik 0/8): O-projection pieces
            for jq, h, ik in stream:
                qsl = slice(QT * jq, QT * (jq + 1))
                ch = h // 2
                if ik == 0:
                    att_ps = ps_att.tile(
                        [HD + 1, QT], F32, tag="att", name=f"att{jq}_{h}"
                    )
                    att_tiles[(jq, h)] = [att_ps, 0]
                if jq > 0:
                    if h == 0 and ik == 12:
                        boundary_q.append(lambda jq=jq: emit_recip(jq - 1))
                    elif h == 1 and ik in (8, 12):
                        boundary_q.append(
                            lambda jq=jq, hh=(ik - 8) // 4: emit_norm_head(jq - 1, hh)
                        )
                    elif h == 2 and ik in (0, 4):
                        boundary_q.append(
                            lambda jq=jq, hh=2 + ik // 4: emit_norm_head(jq - 1, hh)
                        )
                    elif h == 2 and ik in (8, 12):
                        boundary_q.append(
                            lambda jq=jq, dt=(ik - 8) // 4: emit_oproj_dt(jq - 1, dt)
                        )
                    elif h == 3 and ik in (0, 8):
                        boundary_q.append(
                            lambda jq=jq, dt=2 + ik // 8: emit_oproj_dt(jq - 1, dt)
                        )
                if jq == NQT - 1:
                    # last q-tile: per-head direct chains, emitted once their
                    # reciprocal (issued at arm, ~recip-latency earlier) is
                    # ready
                    if h == 2 and ik == 10:
                        boundary_q.append(lambda: finish_direct(NQT - 1, 0))
                    elif h == 3 and ik == 10:
                        boundary_q.append(lambda: finish_direct(NQT - 1, 1))
                if cur["sc"] is None:
                    cur["sc"] = ps_sc.tile([128, SCW], F32, tag="sc", name="sct")
                    cur["pt"] = ptp.tile([128, SCW], BF16, tag="pt", name="ptt")
                j = len(cur["chunks"])
                nc.tensor.matmul(
                    cur["sc"][:, QT * j : QT * (j + 1)],
                    kT_pad[:, h, 128 * ik : 128 * (ik + 1)],
                    qT_sb[:, ch, qsl],
                    start=True,
                    stop=True,
                )
                cur["chunks"].append((jq, h, ik))
                if len(cur["chunks"]) == CH:
                    close_tile()

            close_tile()
            while deferred:
                emit_avs()
            # hold the PE clock up while head 3's reciprocal (issued by the
            # drain above) runs on the DVE
            finish_direct(NQT - 1, 2)
            junk_mm(10, "tailw")
            finish_direct(NQT - 1, 3)
            for dt in range(4):
                emit_oproj_dt(NQT - 1, dt)

    if split_waits:
        _split_excess_waits(nc)
    return nc


_CACHED_NC = None


def _get_nc():
    global _CACHED_NC
    if _CACHED_NC is None:
        _CACHED_NC = build_program()
    return _CACHED_NC


def _sel_mat():
    # sel[p, h*64:(h+1)*64] = 1 iff p == h: broadcasts reciprocal row h
    # across 64 partitions via a contraction-4 matmul.
    s = np.zeros((4, HPG * 64), dtype=np.float32)
    for h in range(HPG):
        s[h, 64 * h : 64 * (h + 1)] = 1.0
    return s


def _bf16(x):
    return np.asarray(x, dtype=ml_dtypes.bfloat16)


def _shard_inputs(query, Wk, Wv, Wo):
    wki = Wk.astype(np.float32) + np.eye(D, dtype=np.float32)
    in_maps = []
    perms = []
    for g in range(HG):
        perm = np.r_[DG * g : DG * (g + 1), 0 : DG * g, DG * (g + 1) : D]
        perms.append(perm)
    for n in range(N):
        for g in range(HG):
            perm = perms[g]
            hg = slice(DG * g, DG * (g + 1))
            qTn = np.ascontiguousarray(query[n].T[perm])          # [512, 2048]
            wkiT = np.ascontiguousarray(wki[hg, :][:, perm].T)    # [512, 256]
            wvT = np.ascontiguousarray(Wv[hg, :][:, perm].T)      # [512, 256]
            woTc = np.ascontiguousarray(Wo[:, hg].T)              # [256, 512]
            woT = np.zeros((HPG * 128, D), dtype=np.float32)
            for hc in range(HPG):
                woT[128 * hc : 128 * hc + 64] = woTc[64 * hc : 64 * hc + 64]
            in_maps.append(
                {
                    "qT": _bf16(qTn),
                    "wkiT": _bf16(wkiT),
                    "wvT": _bf16(wvT),
                    "woT": _bf16(woT),
                    "zeros": np.zeros((64, T), dtype=ml_dtypes.bfloat16),
                    "sel": _bf16(_sel_mat()),
                }
            )
    return in_maps


def run(query, Wk, Wv, Wo, **run_kwargs):
    """Run the SPMD kernel; returns (output, BassKernelResults)."""
    nc = _get_nc()
    in_maps = _shard_inputs(
        np.asarray(query, dtype=np.float32),
        np.asarray(Wk, dtype=np.float32),
        np.asarray(Wv, dtype=np.float32),
        np.asarray(Wo, dtype=np.float32),
    )
    res = run_bass_kernel_spmd(nc, in_maps, list(range(N * HG)), **run_kwargs)
    outs = []
    for n in range(N):
        pT = res.results[2 * n]["outT"] + res.results[2 * n + 1]["outT"]
        outs.append(pT.T)
    return np.stack(outs).astype(np.float32), res


def kernel(query, Wk, Wv, Wo):
    out, _ = run(query, Wk, Wv, Wo)
    return out
